# revision 25
# baseline (speedup 1.0000x reference)
"""ChemConv Bass kernel for 8 TRN2 NeuronCores.

Math: the reference
    node_connection[a,f,i] = sum_n conn[a,n,f] * x[n,i]
    bond_score[a,o,f]      = sum_i node_connection[a,f,i] * pf[o,f,i]
    out[a,o] = sum_f bond_score[a,o,f]*bf[o,f,0] + sum_{f,c} bp[a,f,c]*bf[o,f,1+c]
is computed in "Z-form":
    Z_f[i, a]  = sum_n x[n, i] * conn[a, n, f]     (conn is the streamed moving
                                                    operand; x blocks stationary)
    out[o, a]  = sum_f W2_f[i, o]^T @ Z_f + bond term,  W2[o,f,i] = pf*bf[...,0]

Sharding: atoms (dim a) row-slabs of 256 across 8 cores; x/filters replicated.
Each core streams its 25.2 MB conn slab once (the memory roofline).

Layout: conn is pre-packed host-side to [128, KC*AS] where column
((f*16+nb)*256 + a) holds conn[a, nb*128+p, f] for partition p.  A batch DMA
of B chunks then lands 128 descriptors of B*0.5 KB contiguous per partition
(vs a [K, AS] layout, which is descriptor-overhead-bound at ~190 GB/s).

conn is stored in HBM as fp8 e3m4 of (conn - 0.5): the mean shift halves the
magnitude (one extra bit of effective precision for uniform[0,1) data) and the
PE consumes the fp8 moving operand directly against the bf16 stationary x
(mixed-dtype matmul, verified bit-exact vs numpy on HW).  This halves the
HBM-bound conn stream vs bf16.  The -0.5 shift is exact to correct: it adds
0.5*sum_n x[n,i] to every node_connection[a,f,i], i.e. a constant c[o] per
output column, which is folded in as a 25th row of the bond-term matmul
(bf2 row 24 = c[o], bond row 24 = ones).  Measured end-to-end rel err ~8e-3
(gate 2e-2).  The small phase-2 operands are float32r so the PE runs
single-pass full-rate (plain fp32 lowers to the two-pass LOW/HIGH mode).
Z_f accumulates in PSUM over 16 n-block matmuls; after each f, Z_f is copied
to SBUF and immediately folded into the out accumulator, so the tensor work
trails the conn stream by one chunk and the tail is short.
"""

import ml_dtypes
import numpy as np

import concourse.bass as bass
import concourse.tile as tile
from concourse import bacc, mybir
from concourse.bass_utils import run_bass_kernel_spmd

A = 2048
IN_DEPTH = 64
OUT_DEPTH = 64
F = 12
NCORES = 8
AS = A // NCORES          # 256 atoms per core
KP = 128                  # contraction rows per matmul chunk (partition dim)
NBLK = A // KP            # 16 n-blocks
KC = F * NBLK             # 192 chunks, kc = f*16 + nb (f-major)
K = KC * KP               # 24576 total contraction length
KB = 2 * F + 1            # bond-term contraction length (f,c)=24 + mean-shift row

MM_DT = mybir.dt.float32r  # fp32 bits, full-rate single-pass PE streaming
BF16 = mybir.dt.bfloat16   # x + Z fold operands
F8E3 = mybir.dt.float8e3   # conn stream: e3m4 of (conn - 0.5)
F32 = mybir.dt.float32

_cache = {}


def _build_nc(bufs=24, split_copies=True, colgrp=True):
    """Build the per-core kernel.

    bufs: conn stream-pool buffering depth (deep enough that the stream
        DMAs never wait on a buffer free mid-stream)
    split_copies: split the tail PSUM->SBUF copies across DVE+ACT engines
    colgrp: alternate chunk outputs between PE column groups (PSUM
        partitions 0-63 / 64-127) so each LDWEIGHTS can overlap the
        in-flight matmul of the other group; phase-2 then contracts
        K=128 against a row-doubled W2
    """
    nc = bacc.Bacc("TRN2", target_bir_lowering=False, debug=False)

    conn_t = nc.dram_tensor("conn_t", [KP, KC * AS], F8E3, kind="ExternalInput").ap()
    # bond_t [25, AS] and bf2 [25, O] packed side by side -> one DMA
    bpack = nc.dram_tensor("bpack", [KB, AS + OUT_DEPTH], F32, kind="ExternalInput").ap()
    # x blocks: xpack[p, nb*64+i] = x[nb*128+p, i]
    xpack = nc.dram_tensor("xpack", [KP, NBLK * IN_DEPTH], BF16, kind="ExternalInput").ap()
    # w2[i, f*64+o] = pf[o,f,i] * bf[o,f,0]; row-doubled copy for colgrp mode
    w2 = nc.dram_tensor("w2", [IN_DEPTH, F * OUT_DEPTH], MM_DT, kind="ExternalInput").ap()
    w2d = nc.dram_tensor("w2d", [2 * IN_DEPTH, F * OUT_DEPTH], MM_DT,
                         kind="ExternalInput").ap()
    out_t = nc.dram_tensor("out_t", [OUT_DEPTH, AS], F32, kind="ExternalOutput").ap()

    # conn DMA batches alternate between the two HWDGE rings (SP via nc.sync,
    # ACT via nc.scalar) so each ring's per-batch fixed costs (descriptor gen,
    # HBM completion receipt) overlap the other ring's data movement.  Small
    # head so the PE starts early, tapered tail so the final chunks (which
    # gate the output) aren't stuck behind a large transfer.
    batches = [4, 4] + [8] * 22 + [4, 4]
    assert sum(batches) == KC
    starts = [sum(batches[:i]) for i in range(len(batches))]
    chunk_bt = []
    for bt, bsz in enumerate(batches):
        chunk_bt += [bt] * bsz

    with tile.TileContext(nc) as tc:
        with (
            tc.tile_pool(name="const", bufs=1) as cpool,
            tc.tile_pool(name="stream", bufs=bufs) as spool,
            tc.tile_pool(name="zsb", bufs=3) as zpool,
            tc.tile_pool(name="zpsum", bufs=2, space="PSUM") as zpp,
            tc.tile_pool(name="apsum", bufs=1, space="PSUM") as apool,
            tc.tile_pool(name="warm", bufs=1) as wpool,
            tc.tile_pool(name="wpsum", bufs=1, space="PSUM") as wpp,
        ):
            # HAM warmup: the PE sits idle from the end of the NEFF prologue
            # (~7us) until the first conn batch lands (~11us), and would then
            # run its first ~3.4us of real matmuls at the cold 1.2 GHz clock.
            # A stream of dummy matmuls on a memset tile keeps the PE busy
            # through that window so the HAM unthrottles before real work.
            warm_sb = wpool.tile([KP, IN_DEPTH], BF16)
            nc.gpsimd.memset(warm_sb[:], 0.0)
            warm_ps = wpp.tile([IN_DEPTH, IN_DEPTH], F32, tag="warm")
            for _ in range(40):
                nc.tensor.matmul(warm_ps[:], warm_sb[:], warm_sb[:, :IN_DEPTH],
                                 start=True, stop=True)
            # small input DMAs on the second HWDGE ring (ACT) so the conn
            # stream owns the SP ring from t=0
            x_sb = cpool.tile([KP, NBLK * IN_DEPTH], BF16)
            nc.scalar.dma_start(x_sb[:], xpack[:])
            if colgrp:
                w2_sb = cpool.tile([2 * IN_DEPTH, F * OUT_DEPTH], MM_DT)
                nc.scalar.dma_start(w2_sb[:], w2d[:])
            else:
                w2_sb = cpool.tile([IN_DEPTH, F * OUT_DEPTH], MM_DT)
                nc.scalar.dma_start(w2_sb[:], w2[:])
            bp_sb = cpool.tile([KB, AS + OUT_DEPTH], F32)
            nc.scalar.dma_start(bp_sb[:], bpack[:])
            bond_sb = bp_sb[:, :AS]
            bf2_sb = bp_sb[:, AS:AS + OUT_DEPTH]

            ctiles = {}

            def issue_conn(bt):
                bsz = batches[bt]
                ctile = spool.tile([KP, bsz * AS], F8E3, tag="conn",
                                   name=f"conn_{bt}")
                src = conn_t[:, starts[bt] * AS:(starts[bt] + bsz) * AS]
                # the whole conn stream goes on the SP HWDGE ring: batches on
                # the ACT ring come back corrupted in an a%4 byte-pair pattern
                # (observed on HW with both fp8 and f32-bitcast descriptors),
                # and SWDGE (gpsimd) measured ~6us slower end-to-end
                nc.sync.dma_start(ctile[:], src)
                ctiles[bt] = ctile

            # issue the whole stream upfront: with bufs=8 the rotation waits
            # only bite for the last two (tiny) batches, on queues that have
            # nothing else left to do
            for bt in range(len(batches)):
                issue_conn(bt)

            acc = apool.tile([OUT_DEPTH, AS], F32, tag="acc")
            # bond term opens the out PSUM accumulation group
            nc.tensor.matmul(acc[:], bf2_sb[:], bond_sb[:], start=True, stop=False)

            # phase-2 matmuls are deferred a full f group: the PE queue is
            # in-order, so emitting acc += W2_f^T@Z_f right after f's chunks
            # would stall the PE on the vector copy of Z_f; one group (~1.8us)
            # of deferral gives the copy ample slack.
            pending = []

            def flush_pending(last, keep=0):
                while len(pending) > keep:
                    fp, z = pending.pop(0)
                    nc.tensor.matmul(
                        acc[:],
                        w2_sb[:, fp * OUT_DEPTH:(fp + 1) * OUT_DEPTH],
                        z[:],
                        start=False,
                        stop=last and not pending,
                    )

            # colgrp: even/odd chunks write PSUM partitions 0-63 / 64-127
            # (bass infers tile_position from base partitions), so each
            # LDWEIGHTS targets the column group the in-flight matmul isn't
            # using and can be pulled ahead by the PE reorder window.
            ZP = 2 * IN_DEPTH if colgrp else IN_DEPTH
            for f in range(F):
                zps = zpp.tile([ZP, AS], F32, tag="zps")
                for nb in range(NBLK):
                    kc = f * NBLK + nb
                    bt = chunk_bt[kc]
                    j = kc - starts[bt]
                    if colgrp:
                        g = nb % 2
                        dst = zps[g * IN_DEPTH:(g + 1) * IN_DEPTH, :]
                        st, sp = nb < 2, nb >= NBLK - 2
                    else:
                        dst = zps[:]
                        st, sp = nb == 0, nb == NBLK - 1
                    # skip_group_check: CoreSim's zero-region tracker loses the
                    # base partition of the colgrp halves and false-positives;
                    # no effect on hardware.
                    nc.tensor.matmul(
                        dst,
                        x_sb[:, nb * IN_DEPTH:(nb + 1) * IN_DEPTH],
                        ctiles[bt][:, j * AS:(j + 1) * AS],
                        start=st,
                        stop=sp,
                        skip_group_check=colgrp,
                    )
                    if nb == 2:
                        flush_pending(False, keep=1)
                # split the PSUM->SBUF copy across DVE and ACT (a full-width
                # DVE copy of [128,256] PSUM fp32 was observed to corrupt
                # alternating 8-byte pairs on HW; the split form is reliable
                # and also halves the copy latency on the tail path)
                z_sb = zpool.tile([ZP, AS], MM_DT, tag="z", name=f"z_{f}")
                if split_copies:
                    h = AS // 2
                    nc.vector.tensor_copy(z_sb[:, :h], zps[:, :h].bitcast(MM_DT))
                    nc.scalar.copy(z_sb[:, h:], zps[:, h:].bitcast(MM_DT))
                else:
                    nc.vector.tensor_copy(z_sb[:], zps[:].bitcast(MM_DT))
                pending.append((f, z_sb))
            flush_pending(True)

            # tail: copy + store the two output halves on separate engine
            # pairs so the copies and the DMAs overlap
            out_sb = spool.tile([OUT_DEPTH, AS], F32, tag="osb", bufs=1)
            if split_copies:
                h = AS // 2
                nc.vector.tensor_copy(out_sb[:, :h], acc[:, :h])
                nc.scalar.copy(out_sb[:, h:], acc[:, h:])
                nc.sync.dma_start(out_t[:, :h], out_sb[:, :h])
                nc.scalar.dma_start(out_t[:, h:], out_sb[:, h:])
            else:
                nc.vector.tensor_copy(out_sb[:], acc[:])
                nc.sync.dma_start(out_t[:], out_sb[:])

    nc.compile()
    return nc


def _prep(node_property_tensor, connectivity_tensor, bond_property_tensor,
          property_filters, bond_filters):
    x = np.asarray(node_property_tensor, dtype=np.float32)
    conn = np.asarray(connectivity_tensor, dtype=np.float32)
    bp = np.asarray(bond_property_tensor, dtype=np.float32)
    pf = np.asarray(property_filters, dtype=np.float32)
    bf = np.asarray(bond_filters, dtype=np.float32)

    W = pf * bf[:, :, 0:1]                                # (O, F, I)
    w2 = np.ascontiguousarray(W.transpose(2, 1, 0).reshape(IN_DEPTH, F * OUT_DEPTH))
    bf2 = bf[:, :, 1:3].reshape(OUT_DEPTH, 2 * F).T      # (24, O)
    # conn is stored mean-shifted by -0.5; the missing 0.5*sum_n x[n,i] term
    # contributes a constant c[o] per output column, folded in as an extra
    # contraction row of the bond matmul (against a row of ones).
    c = 0.5 * np.einsum('i,ofi->o', x.sum(axis=0), W)    # (O,)
    bf2 = np.ascontiguousarray(np.concatenate([bf2, c[None, :]], axis=0))  # (25, O)
    xpack = np.ascontiguousarray(
        x.reshape(NBLK, KP, IN_DEPTH).transpose(1, 0, 2).reshape(KP, NBLK * IN_DEPTH)
    ).astype(ml_dtypes.bfloat16)

    common = {"xpack": xpack, "w2": w2,
              "w2d": np.ascontiguousarray(np.concatenate([w2, w2], axis=0))}
    in_maps = []
    ones_row = np.ones((1, AS), dtype=np.float32)
    for ci in range(NCORES):
        sl = slice(ci * AS, (ci + 1) * AS)
        # conn_t[p, (f*16+nb)*256 + a] = conn[a0+a, nb*128+p, f] - 0.5, in e3m4
        cslab = (conn[sl] - 0.5).astype(ml_dtypes.float8_e3m4).reshape(
            AS, NBLK, KP, F)
        conn_c = np.ascontiguousarray(
            cslab.transpose(2, 3, 1, 0).reshape(KP, KC * AS))
        bond_tc = np.concatenate(
            [bp[sl].reshape(AS, 2 * F).T, ones_row], axis=0)  # (25, AS)
        in_maps.append({
            "conn_t": conn_c,
            "bpack": np.ascontiguousarray(
                np.concatenate([bond_tc, bf2], axis=1)),  # (25, AS + O)
            **common,
        })
    return in_maps


def kernel(node_property_tensor, connectivity_tensor, bond_property_tensor,
           property_filters, bond_filters):
    in_maps = _prep(node_property_tensor, connectivity_tensor,
                    bond_property_tensor, property_filters, bond_filters)

    if "nc" not in _cache:
        _cache["nc"] = _build_nc()
    nc = _cache["nc"]

    res = run_bass_kernel_spmd(nc, in_maps, core_ids=list(range(NCORES)))

    out = np.empty((A, OUT_DEPTH), dtype=np.float32)
    for c in range(NCORES):
        out[c * AS:(c + 1) * AS, :] = res.results[c]["out_t"].T
    return out



# revision 26
# speedup vs baseline: 1.0092x; 1.0092x over previous
"""ChemConv Bass kernel for 8 TRN2 NeuronCores.

Math: the reference
    node_connection[a,f,i] = sum_n conn[a,n,f] * x[n,i]
    bond_score[a,o,f]      = sum_i node_connection[a,f,i] * pf[o,f,i]
    out[a,o] = sum_f bond_score[a,o,f]*bf[o,f,0] + sum_{f,c} bp[a,f,c]*bf[o,f,1+c]
is computed in "Z-form":
    Z_f[i, a]  = sum_n x[n, i] * conn[a, n, f]     (conn is the streamed moving
                                                    operand; x blocks stationary)
    out[o, a]  = sum_f W2_f[i, o]^T @ Z_f + bond term,  W2[o,f,i] = pf*bf[...,0]

Sharding: atoms (dim a) row-slabs of 256 across 8 cores; x/filters replicated.
Each core streams its 25.2 MB conn slab once (the memory roofline).

Layout: conn is pre-packed host-side to [128, KC*AS] where column
((f*16+nb)*256 + a) holds conn[a, nb*128+p, f] for partition p.  A batch DMA
of B chunks then lands 128 descriptors of B*0.5 KB contiguous per partition
(vs a [K, AS] layout, which is descriptor-overhead-bound at ~190 GB/s).

conn is stored in HBM as fp8 e3m4 of (conn - 0.5): the mean shift halves the
magnitude (one extra bit of effective precision for uniform[0,1) data) and the
PE consumes the fp8 moving operand directly against the bf16 stationary x
(mixed-dtype matmul, verified bit-exact vs numpy on HW).  This halves the
HBM-bound conn stream vs bf16.  The -0.5 shift is exact to correct: it adds
0.5*sum_n x[n,i] to every node_connection[a,f,i], i.e. a constant c[o] per
output column, which is folded in as a 25th row of the bond-term matmul
(bf2 row 24 = c[o], bond row 24 = ones).  Measured end-to-end rel err ~8e-3
(gate 2e-2).  The small phase-2 operands are float32r so the PE runs
single-pass full-rate (plain fp32 lowers to the two-pass LOW/HIGH mode).
Z_f accumulates in PSUM over 16 n-block matmuls; after each f, Z_f is copied
to SBUF and immediately folded into the out accumulator, so the tensor work
trails the conn stream by one chunk and the tail is short.
"""

import ml_dtypes
import numpy as np

import concourse.bass as bass
import concourse.tile as tile
from concourse import bacc, mybir
from concourse.bass_utils import run_bass_kernel_spmd

A = 2048
IN_DEPTH = 64
OUT_DEPTH = 64
F = 12
NCORES = 8
AS = A // NCORES          # 256 atoms per core
KP = 128                  # contraction rows per matmul chunk (partition dim)
NBLK = A // KP            # 16 n-blocks
KC = F * NBLK             # 192 chunks, kc = f*16 + nb (f-major)
K = KC * KP               # 24576 total contraction length
KB = 2 * F + 1            # bond-term contraction length (f,c)=24 + mean-shift row

MM_DT = mybir.dt.float32r  # fp32 bits, full-rate single-pass PE streaming
BF16 = mybir.dt.bfloat16   # x + Z fold operands
F8E3 = mybir.dt.float8e3   # conn stream: e3m4 of (conn - 0.5)
F32 = mybir.dt.float32

_cache = {}


def _build_nc(bufs=24, split_copies=True, colgrp=True):
    """Build the per-core kernel.

    bufs: conn stream-pool buffering depth (deep enough that the stream
        DMAs never wait on a buffer free mid-stream)
    split_copies: split the tail PSUM->SBUF copies across DVE+ACT engines
    colgrp: alternate chunk outputs between PE column groups (PSUM
        partitions 0-63 / 64-127) so each LDWEIGHTS can overlap the
        in-flight matmul of the other group; phase-2 then contracts
        K=128 against a row-doubled W2
    """
    nc = bacc.Bacc("TRN2", target_bir_lowering=False, debug=False)

    conn_t = nc.dram_tensor("conn_t", [KP, KC * AS], F8E3, kind="ExternalInput").ap()
    # bond_t [25, AS] and bf2 [25, O] packed side by side -> one DMA
    bpack = nc.dram_tensor("bpack", [KB, AS + OUT_DEPTH], F32, kind="ExternalInput").ap()
    # x blocks: xpack[p, nb*64+i] = x[nb*128+p, i]
    xpack = nc.dram_tensor("xpack", [KP, NBLK * IN_DEPTH], BF16, kind="ExternalInput").ap()
    # w2[i, f*64+o] = pf[o,f,i] * bf[o,f,0]; row-doubled copy for colgrp mode
    w2 = nc.dram_tensor("w2", [IN_DEPTH, F * OUT_DEPTH], MM_DT, kind="ExternalInput").ap()
    w2d = nc.dram_tensor("w2d", [2 * IN_DEPTH, F * OUT_DEPTH], MM_DT,
                         kind="ExternalInput").ap()
    out_t = nc.dram_tensor("out_t", [OUT_DEPTH, AS], F32, kind="ExternalOutput").ap()

    # conn DMA batches alternate between the two HWDGE rings (SP via nc.sync,
    # ACT via nc.scalar) so each ring's per-batch fixed costs (descriptor gen,
    # HBM completion receipt) overlap the other ring's data movement.  Small
    # head so the PE starts early, tapered tail so the final chunks (which
    # gate the output) aren't stuck behind a large transfer.
    batches = [4, 4] + [8] * 22 + [4, 4]
    assert sum(batches) == KC
    starts = [sum(batches[:i]) for i in range(len(batches))]
    chunk_bt = []
    for bt, bsz in enumerate(batches):
        chunk_bt += [bt] * bsz

    with tile.TileContext(nc) as tc:
        with (
            tc.tile_pool(name="const", bufs=1) as cpool,
            tc.tile_pool(name="stream", bufs=bufs) as spool,
            tc.tile_pool(name="zsb", bufs=3) as zpool,
            tc.tile_pool(name="zpsum", bufs=2, space="PSUM") as zpp,
            tc.tile_pool(name="apsum", bufs=1, space="PSUM") as apool,
            tc.tile_pool(name="warm", bufs=1) as wpool,
            tc.tile_pool(name="wpsum", bufs=1, space="PSUM") as wpp,
        ):
            # HAM warmup: the PE sits idle from the end of the NEFF prologue
            # (~7us) until the first conn batch lands (~11us), and would then
            # run its first ~3.4us of real matmuls at the cold 1.2 GHz clock.
            # A stream of dummy matmuls on a memset tile keeps the PE busy
            # through that window so the HAM unthrottles before real work.
            warm_sb = wpool.tile([KP, IN_DEPTH], BF16)
            nc.gpsimd.memset(warm_sb[:], 0.0)
            # ~72 x 53ns spans the full 3.4us HAM SHORT window between the
            # prologue end and the first conn batch landing
            warm_ps = wpp.tile([IN_DEPTH, IN_DEPTH], F32, tag="warm")
            for _ in range(72):
                nc.tensor.matmul(warm_ps[:], warm_sb[:], warm_sb[:, :IN_DEPTH],
                                 start=True, stop=True)
            # small input DMAs on the second HWDGE ring (ACT) so the conn
            # stream owns the SP ring from t=0
            x_sb = cpool.tile([KP, NBLK * IN_DEPTH], BF16)
            nc.scalar.dma_start(x_sb[:], xpack[:])
            if colgrp:
                w2_sb = cpool.tile([2 * IN_DEPTH, F * OUT_DEPTH], MM_DT)
                nc.scalar.dma_start(w2_sb[:], w2d[:])
            else:
                w2_sb = cpool.tile([IN_DEPTH, F * OUT_DEPTH], MM_DT)
                nc.scalar.dma_start(w2_sb[:], w2[:])
            bp_sb = cpool.tile([KB, AS + OUT_DEPTH], F32)
            nc.scalar.dma_start(bp_sb[:], bpack[:])
            bond_sb = bp_sb[:, :AS]
            bf2_sb = bp_sb[:, AS:AS + OUT_DEPTH]

            ctiles = {}

            def issue_conn(bt):
                bsz = batches[bt]
                ctile = spool.tile([KP, bsz * AS], F8E3, tag="conn",
                                   name=f"conn_{bt}")
                src = conn_t[:, starts[bt] * AS:(starts[bt] + bsz) * AS]
                # the whole conn stream goes on the SP HWDGE ring: batches on
                # the ACT ring come back corrupted in an a%4 byte-pair pattern
                # (observed on HW with both fp8 and f32-bitcast descriptors),
                # and SWDGE (gpsimd) measured ~6us slower end-to-end
                nc.sync.dma_start(ctile[:], src)
                ctiles[bt] = ctile

            # issue the whole stream upfront: with bufs=8 the rotation waits
            # only bite for the last two (tiny) batches, on queues that have
            # nothing else left to do
            for bt in range(len(batches)):
                issue_conn(bt)

            acc = apool.tile([OUT_DEPTH, AS], F32, tag="acc")
            # bond term opens the out PSUM accumulation group
            nc.tensor.matmul(acc[:], bf2_sb[:], bond_sb[:], start=True, stop=False)

            # phase-2 matmuls are deferred a full f group: the PE queue is
            # in-order, so emitting acc += W2_f^T@Z_f right after f's chunks
            # would stall the PE on the vector copy of Z_f; one group (~1.8us)
            # of deferral gives the copy ample slack.
            pending = []

            def flush_pending(last, keep=0):
                while len(pending) > keep:
                    fp, z = pending.pop(0)
                    nc.tensor.matmul(
                        acc[:],
                        w2_sb[:, fp * OUT_DEPTH:(fp + 1) * OUT_DEPTH],
                        z[:],
                        start=False,
                        stop=last and not pending,
                    )

            # colgrp: even/odd chunks write PSUM partitions 0-63 / 64-127
            # (bass infers tile_position from base partitions), so each
            # LDWEIGHTS targets the column group the in-flight matmul isn't
            # using and can be pulled ahead by the PE reorder window.
            ZP = 2 * IN_DEPTH if colgrp else IN_DEPTH
            for f in range(F):
                zps = zpp.tile([ZP, AS], F32, tag="zps")
                for nb in range(NBLK):
                    kc = f * NBLK + nb
                    bt = chunk_bt[kc]
                    j = kc - starts[bt]
                    if colgrp:
                        g = nb % 2
                        dst = zps[g * IN_DEPTH:(g + 1) * IN_DEPTH, :]
                        st, sp = nb < 2, nb >= NBLK - 2
                    else:
                        dst = zps[:]
                        st, sp = nb == 0, nb == NBLK - 1
                    # skip_group_check: CoreSim's zero-region tracker loses the
                    # base partition of the colgrp halves and false-positives;
                    # no effect on hardware.
                    nc.tensor.matmul(
                        dst,
                        x_sb[:, nb * IN_DEPTH:(nb + 1) * IN_DEPTH],
                        ctiles[bt][:, j * AS:(j + 1) * AS],
                        start=st,
                        stop=sp,
                        skip_group_check=colgrp,
                    )
                    if nb == 2:
                        flush_pending(False, keep=1)
                # split the PSUM->SBUF copy across DVE and ACT (a full-width
                # DVE copy of [128,256] PSUM fp32 was observed to corrupt
                # alternating 8-byte pairs on HW; the split form is reliable
                # and also halves the copy latency on the tail path)
                z_sb = zpool.tile([ZP, AS], MM_DT, tag="z", name=f"z_{f}")
                if split_copies:
                    h = AS // 2
                    nc.vector.tensor_copy(z_sb[:, :h], zps[:, :h].bitcast(MM_DT))
                    nc.scalar.copy(z_sb[:, h:], zps[:, h:].bitcast(MM_DT))
                else:
                    nc.vector.tensor_copy(z_sb[:], zps[:].bitcast(MM_DT))
                pending.append((f, z_sb))
            flush_pending(True)

            # tail: copy + store the two output halves on separate engine
            # pairs so the copies and the DMAs overlap
            out_sb = spool.tile([OUT_DEPTH, AS], F32, tag="osb", bufs=1)
            if split_copies:
                h = AS // 2
                nc.vector.tensor_copy(out_sb[:, :h], acc[:, :h])
                nc.scalar.copy(out_sb[:, h:], acc[:, h:])
                nc.sync.dma_start(out_t[:, :h], out_sb[:, :h])
                nc.scalar.dma_start(out_t[:, h:], out_sb[:, h:])
            else:
                nc.vector.tensor_copy(out_sb[:], acc[:])
                nc.sync.dma_start(out_t[:], out_sb[:])

    nc.compile()
    return nc


def _prep(node_property_tensor, connectivity_tensor, bond_property_tensor,
          property_filters, bond_filters):
    x = np.asarray(node_property_tensor, dtype=np.float32)
    conn = np.asarray(connectivity_tensor, dtype=np.float32)
    bp = np.asarray(bond_property_tensor, dtype=np.float32)
    pf = np.asarray(property_filters, dtype=np.float32)
    bf = np.asarray(bond_filters, dtype=np.float32)

    W = pf * bf[:, :, 0:1]                                # (O, F, I)
    w2 = np.ascontiguousarray(W.transpose(2, 1, 0).reshape(IN_DEPTH, F * OUT_DEPTH))
    bf2 = bf[:, :, 1:3].reshape(OUT_DEPTH, 2 * F).T      # (24, O)
    # conn is stored mean-shifted by -0.5; the missing 0.5*sum_n x[n,i] term
    # contributes a constant c[o] per output column, folded in as an extra
    # contraction row of the bond matmul (against a row of ones).
    c = 0.5 * np.einsum('i,ofi->o', x.sum(axis=0), W)    # (O,)
    bf2 = np.ascontiguousarray(np.concatenate([bf2, c[None, :]], axis=0))  # (25, O)
    xpack = np.ascontiguousarray(
        x.reshape(NBLK, KP, IN_DEPTH).transpose(1, 0, 2).reshape(KP, NBLK * IN_DEPTH)
    ).astype(ml_dtypes.bfloat16)

    common = {"xpack": xpack, "w2": w2,
              "w2d": np.ascontiguousarray(np.concatenate([w2, w2], axis=0))}
    in_maps = []
    ones_row = np.ones((1, AS), dtype=np.float32)
    for ci in range(NCORES):
        sl = slice(ci * AS, (ci + 1) * AS)
        # conn_t[p, (f*16+nb)*256 + a] = conn[a0+a, nb*128+p, f] - 0.5, in e3m4
        cslab = (conn[sl] - 0.5).astype(ml_dtypes.float8_e3m4).reshape(
            AS, NBLK, KP, F)
        conn_c = np.ascontiguousarray(
            cslab.transpose(2, 3, 1, 0).reshape(KP, KC * AS))
        bond_tc = np.concatenate(
            [bp[sl].reshape(AS, 2 * F).T, ones_row], axis=0)  # (25, AS)
        in_maps.append({
            "conn_t": conn_c,
            "bpack": np.ascontiguousarray(
                np.concatenate([bond_tc, bf2], axis=1)),  # (25, AS + O)
            **common,
        })
    return in_maps


def kernel(node_property_tensor, connectivity_tensor, bond_property_tensor,
           property_filters, bond_filters):
    in_maps = _prep(node_property_tensor, connectivity_tensor,
                    bond_property_tensor, property_filters, bond_filters)

    if "nc" not in _cache:
        _cache["nc"] = _build_nc()
    nc = _cache["nc"]

    res = run_bass_kernel_spmd(nc, in_maps, core_ids=list(range(NCORES)))

    out = np.empty((A, OUT_DEPTH), dtype=np.float32)
    for c in range(NCORES):
        out[c * AS:(c + 1) * AS, :] = res.results[c]["out_t"].T
    return out

